# revision 1
# baseline (speedup 1.0000x reference)
"""v2 kernel reconstruction (for A/B device-state control runs).

bf16 operands, x resident as 32 per-(lb,hb) tiles, JIT-ordered DMA,
fused stt elementwise, accum_out sigmoid, fp32r rowsum, yp bufs=4.
"""

import numpy as np
import ml_dtypes

import concourse.bass as bass  # noqa: F401
import concourse.tile as tile
from concourse import bacc, mybir
from concourse.bass_utils import run_bass_kernel_spmd

dt = mybir.dt
AF = mybir.ActivationFunctionType
ALU = mybir.AluOpType

N, L, H = 8, 2048, 1024
P = 128
LB = 512
NH = H // P
NL = L // LB
N_CORES = 8
NC = NH + 1 + NH + NH

_CACHE = {}


def _build():
    nc = bacc.Bacc("TRN2", target_bir_lowering=False, debug=False,
                   num_devices=N_CORES)

    xT_d = nc.dram_tensor("xT", [H, L], dt.bfloat16, kind="ExternalInput").ap()
    MT_d = nc.dram_tensor("MT", [NH, P, NH * P], dt.bfloat16,
                          kind="ExternalInput").ap()
    WcT_d = nc.dram_tensor("WcT", [NH, P, NH * P], dt.bfloat16,
                           kind="ExternalInput").ap()
    cp_d = nc.dram_tensor("cpack", [P, NC], dt.float32, kind="ExternalInput").ap()
    ones_d = nc.dram_tensor("ones", [P, P], dt.float32r,
                            kind="ExternalInput").ap()
    out_d = nc.dram_tensor("outT", [H, L], dt.float32, kind="ExternalOutput").ap()

    xT3 = xT_d.rearrange("(j p) l -> p j l", p=P)

    with tile.TileContext(nc) as tc:
        with (
            tc.tile_pool(name="resident", bufs=1) as rp,
            tc.tile_pool(name="weights", bufs=1) as wtp,
            tc.tile_pool(name="work", bufs=3) as wp,
            tc.tile_pool(name="mmpsum", bufs=4, space="PSUM") as yp,
            tc.tile_pool(name="dpsum", bufs=2, space="PSUM") as dp,
        ):
            t_s = rp.tile([P, L], dt.float32)
            cs = rp.tile([P, L], dt.float32)

            def load_w(src3, ob, tag, eng):
                t = wtp.tile([P, NH * P], dt.bfloat16, tag=f"{tag}{ob}")
                eng.dma_start(t[:], src3[ob])
                return t

            def load_xb(lb, hb, eng):
                t = wtp.tile([P, LB], dt.bfloat16, tag=f"xb{lb}_{hb}")
                eng.dma_start(t[:], xT3[:, hb, lb * LB:(lb + 1) * LB])
                return t

            xbs = {}
            mt0c0 = wtp.tile([P, P], dt.bfloat16, tag="mt0c0")
            nc.scalar.dma_start(mt0c0[:], MT_d[0, :, 0:P])
            xbs[(0, 0)] = load_xb(0, 0, nc.sync)
            cp = rp.tile([P, NC], dt.float32)
            nc.sync.dma_start(cp[:], cp_d[:])
            mt0c1 = wtp.tile([P, 3 * P], dt.bfloat16, tag="mt0c1")
            nc.gpsimd.dma_start(mt0c1[:], MT_d[0, :, P:4 * P])
            mt0c2 = wtp.tile([P, 4 * P], dt.bfloat16, tag="mt0c2")
            nc.gpsimd.dma_start(mt0c2[:], MT_d[0, :, 4 * P:8 * P])
            for hb in range(1, NH):
                xbs[(0, hb)] = load_xb(0, hb, nc.scalar if hb % 2 else nc.sync)

            ub = cp[:, :NH]
            c0b = cp[:, NH:NH + 1]
            bcb = cp[:, NH + 1:NH + 1 + NH]
            bob = cp[:, NH + 1 + NH:]

            mt = [None] * NH
            wct = [None] * NH
            mt[1] = load_w(MT_d, 1, "mt", nc.gpsimd)
            ones = rp.tile([P, P], dt.float32r)
            nc.gpsimd.dma_start(ones[:], ones_d[:])
            mt[2] = load_w(MT_d, 2, "mt", nc.scalar)
            mt[3] = load_w(MT_d, 3, "mt", nc.gpsimd)
            for hb in range(NH):
                xbs[(1, hb)] = load_xb(1, hb, nc.scalar if hb % 2 else nc.sync)
            for ob in range(4, NH):
                mt[ob] = load_w(MT_d, ob, "mt", nc.gpsimd)
            for ob in range(NH):
                wct[ob] = load_w(WcT_d, ob, "wct", nc.gpsimd)

            def mt0_ap(hb):
                if hb == 0:
                    return mt0c0[:]
                if hb < 4:
                    return mt0c1[:, (hb - 1) * P:hb * P]
                return mt0c2[:, (hb - 4) * P:(hb - 3) * P]

            def mt_ap(ob, hb):
                if ob == 0:
                    return mt0_ap(hb)
                return mt[ob][:, hb * P:(hb + 1) * P]

            sp = [rp.tile([P, 1], dt.float32, name=f"sp{i}", tag=f"sp{i}")
                  for i in range(NL)]

            state = {"pending": None}

            def flush_pending():
                if state["pending"] is None:
                    return
                pd_t, prod_t, lb = state["pending"]
                state["pending"] = None
                nc.tensor.matmul(pd_t[:], ones[:], prod_t[:],
                                 start=True, stop=True)
                ls = slice(lb * LB, (lb + 1) * LB)
                nc.scalar.activation(t_s[:, ls], pd_t[:], AF.Sigmoid,
                                     bias=c0b[:, 0:1], scale=-1.0,
                                     accum_out=sp[lb][:])

            for lb in range(NL):
                pd = dp.tile([P, LB], dt.float32)
                acc = None
                for ob in range(NH):
                    py = yp.tile([P, LB], dt.float32, tag="mm")
                    for hb in range(NH):
                        nc.tensor.matmul(
                            py[:], mt_ap(ob, hb), xbs[(lb, hb)][:],
                            start=(hb == 0), stop=(hb == NH - 1))
                    if ob == 1:
                        flush_pending()
                    prod = wp.tile([P, LB], dt.float32r, tag="prod")
                    nc.vector.scalar_tensor_tensor(
                        prod[:], py[:], ub[:, ob:ob + 1], xbs[(lb, ob)][:],
                        op0=ALU.add, op1=ALU.mult)
                    if acc is None:
                        acc = prod
                    else:
                        nacc = wp.tile([P, LB], dt.float32r, tag="pacc")
                        nc.vector.tensor_tensor(nacc[:], acc[:], prod[:],
                                                op=ALU.add)
                        acc = nacc
                state["pending"] = (pd, acc, lb)
                nxt = lb + 2
                if nxt < NL:
                    for hb in range(NH):
                        xbs[(nxt, hb)] = load_xb(
                            nxt, hb, nc.scalar if hb % 2 else nc.sync)

            for lb in range(NL):
                for ob in range(NH):
                    last_grp = (lb == NL - 1 and ob == NH - 1)
                    nmm = 2 if last_grp else 1
                    mw = LB // nmm
                    pzs = []
                    for ck in range(nmm):
                        pz = yp.tile([P, mw], dt.float32, tag="mm")
                        for hb in range(NH):
                            nc.tensor.matmul(
                                pz[:], wct[ob][:, hb * P:(hb + 1) * P],
                                xbs[(lb, hb)][:, ck * mw:(ck + 1) * mw],
                                start=(hb == 0), stop=(hb == NH - 1))
                        pzs.append(pz)
                    if lb == 0 and ob == 0:
                        flush_pending()
                        s01 = rp.tile([P, 1], dt.float32)
                        nc.vector.tensor_tensor(s01[:], sp[0][:], sp[1][:],
                                                op=ALU.add)
                        s23 = rp.tile([P, 1], dt.float32)
                        nc.vector.tensor_tensor(s23[:], sp[2][:], sp[3][:],
                                                op=ALU.add)
                        s_all = rp.tile([P, 1], dt.float32)
                        nc.vector.tensor_tensor(s_all[:], s01[:], s23[:],
                                                op=ALU.add)
                        S1_t = rp.tile([P, 1], dt.float32)
                        nc.vector.tensor_scalar(
                            S1_t[:], s_all[:], 1.0 / (L - 1), 1.0,
                            op0=ALU.mult, op1=ALU.add)
                        nc.vector.tensor_scalar(
                            cs[:], t_s[:], -float(L) / (L - 1), S1_t[:],
                            op0=ALU.mult, op1=ALU.add)
                    for ck in range(nmm):
                        lo = lb * LB + ck * mw
                        lsc = slice(lo, lo + mw)
                        zc = wp.tile([P, mw], dt.float32, tag="zc")
                        nc.vector.scalar_tensor_tensor(
                            zc[:], pzs[ck][:], bcb[:, ob:ob + 1], cs[:, lsc],
                            op0=ALU.add, op1=ALU.mult)
                        ot = wp.tile([P, mw], dt.float32, tag="ot")
                        nc.vector.tensor_scalar_add(
                            ot[:], zc[:], bob[:, ob:ob + 1])
                        eng = nc.scalar if (ob + ck) % 2 else nc.sync
                        eng.dma_start(out_d[ob * P:(ob + 1) * P, lsc], ot[:])

    nc.compile()
    return nc


def _get_nc():
    if "nc" not in _CACHE:
        _CACHE["nc"] = _build()
    return _CACHE["nc"]


def _prep_inputs(x, Wq, bq, Wk, bk, Wv, bv, Wo, bo):
    f8 = np.float64
    bf = ml_dtypes.bfloat16
    M = (Wq.astype(f8).T @ Wk.astype(f8)).astype(np.float32)
    u = (Wk.astype(f8).T @ bq.astype(f8)
         + Wq.astype(f8).T @ bk.astype(f8)).astype(np.float32)
    c0 = np.float32(bq.astype(f8) @ bk.astype(f8))
    Wc = (Wo.astype(f8) @ Wv.astype(f8)).astype(np.float32)
    bc = (Wo.astype(f8) @ bv.astype(f8)).astype(np.float32)

    def _pack(WT):
        t = WT.reshape(NH, P, NH, P)
        return np.ascontiguousarray(
            t.transpose(2, 1, 0, 3).reshape(NH, P, NH * P).astype(bf))

    MT = _pack(M.T)
    WcT = _pack(Wc.T)
    ub = u.reshape(NH, P).T
    bcb = bc.reshape(NH, P).T
    bob = bo.astype(np.float32).reshape(NH, P).T
    c0b = np.full((P, 1), np.log(L - 1.0) - np.float64(c0), np.float32)
    cpack = np.ascontiguousarray(
        np.concatenate([ub, c0b, bcb, bob], axis=1).astype(np.float32))
    ones = np.ones((P, P), np.float32)

    shared = dict(MT=MT, WcT=WcT, cpack=cpack, ones=ones)
    in_maps = []
    for n in range(N_CORES):
        xT = np.ascontiguousarray(x[n].T.astype(bf))
        in_maps.append(dict(xT=xT, **shared))
    return in_maps


def kernel(x, Wq, bq, Wk, bk, Wv, bv, Wo, bo, _trace=False, _trace_kwargs=None):
    x, Wq, bq, Wk, bk, Wv, bv, Wo, bo = (
        np.asarray(a) for a in (x, Wq, bq, Wk, bk, Wv, bv, Wo, bo))
    nc = _get_nc()
    in_maps = _prep_inputs(x, Wq, bq, Wk, bk, Wv, bv, Wo, bo)
    res = run_bass_kernel_spmd(nc, in_maps, list(range(N_CORES)),
                               trace=_trace, **(_trace_kwargs or {}))
    out = np.empty((N, L, H), np.float32)
    for n in range(N_CORES):
        out[n] = res.results[n]["outT"].T
    if _trace:
        kernel.last_result = res
    return out

